# revision 25
# baseline (speedup 1.0000x reference)
"""Trainium2 Bass kernel for the ELGCA block (dwconv3x3+gelu || conv1x1+gelu
-> pooled linear attention), data-parallel over batch on 8 NeuronCores.

Self-contained: hardcodes shapes B=16, C=128, H=W=128, f32.
kernel(**inputs) takes full unsharded inputs, returns the FULL output.

Per-core layout (BPC=2 local images b0,b1), partitions p=(b*64+c):
  dwconv3x3: 9 accumulating PE matmuls per 512-col chunk; lhsT = diagonal
             [128x128] of tap weights (fp32r), rhs = flat-shifted window of
             the zero-padded x1 strip (fp32r).  PSUM holds the 9-tap sum in
             f32; ACT applies bias+gelu on evacuation.
  conv1x1:   block-diagonal fp32r matmuls per 512-col chunk;
             A psum = [q(b0)|q(b1)|k(b0)|k(b1)], B = [v(b0)|v(b1)|l(b0)|l(b1)].
  attention: pooled qf/kf -> per-quarter packed transposes -> qk accumulated
             in one [32,64] PSUM bank -> softmax stats -> packed out2 matmul.
"""

import ml_dtypes
import numpy as np
from contextlib import ExitStack

import concourse.bass as bass
import concourse.tile as tile
from concourse import bacc, mybir
from concourse import bass_utils
from concourse.masks import make_identity

F32 = mybir.dt.float32
F32R = mybir.dt.float32r
BF16 = mybir.dt.bfloat16
AX = mybir.AxisListType
ALU = mybir.AluOpType
ACTF = mybir.ActivationFunctionType

N_CORES = 8
B_TOT, C, H, W = 16, 128, 128, 128
BPC = B_TOT // N_CORES          # 2 images per core
HW = H * W                      # 16384
C2 = C // 2                     # 64
C4 = C // 4                     # 32
WP = W + 2                      # padded row width for dwconv
R = 16                          # dwconv row-strip height
NCH = 512                       # conv1x1 / out2 column chunk
NCHUNKS = HW // NCH             # 32
NSTRIPS = H // R                # 8
RPC = NCH // W                  # image rows per 512-col chunk (4)
NP = (H // 2) * (W // 2)        # 4096 pooled positions

DVE_STRIPS = ()

# dwconv taps in row-major (dy, dx) order
TAPS = [(dy, dx) for dy in (-1, 0, 1) for dx in (-1, 0, 1)]


def build_nc(loops=1):
    nc = bacc.Bacc("TRN2", target_bir_lowering=False, debug=False,
                   num_devices=N_CORES)
    x = nc.dram_tensor("x", [BPC, C, H, W], F32, kind="ExternalInput").ap()
    d_lhsA = nc.dram_tensor("lhsA", [128, 128], F32,
                            kind="ExternalInput").ap()
    d_lhsB = nc.dram_tensor("lhsB", [128, 128], BF16,
                            kind="ExternalInput").ap()
    d_biases = nc.dram_tensor("biases", [128, 3], F32,
                              kind="ExternalInput").ap()
    d_dwdiag = nc.dram_tensor("dwdiag", [128, 9 * 128], BF16,
                              kind="ExternalInput").ap()
    d_dww9 = nc.dram_tensor("dww9", [128, 9], F32,
                            kind="ExternalInput").ap()
    out = nc.dram_tensor("out", [BPC, C, H, W], F32, kind="ExternalOutput").ap()

    xflat = x.rearrange("b c h w -> b c (h w)")
    oflat = out.rearrange("b c h w -> b c (h w)")

    with tile.TileContext(nc) as tc, ExitStack() as ctx:
        consts = ctx.enter_context(tc.tile_pool(name="consts", bufs=1))
        x1p = ctx.enter_context(tc.tile_pool(name="x1p", bufs=2))
        rhsp = ctx.enter_context(tc.tile_pool(name="rhsp", bufs=2))
        packp = ctx.enter_context(tc.tile_pool(name="packp", bufs=1))
        poolt = ctx.enter_context(tc.tile_pool(name="poolt", bufs=1))
        attnp = ctx.enter_context(tc.tile_pool(name="attnp", bufs=1))
        stgp = ctx.enter_context(tc.tile_pool(name="stgp", bufs=1))
        ps_convA = ctx.enter_context(
            tc.tile_pool(name="ps_convA", bufs=2, space="PSUM"))
        ps_convB = ctx.enter_context(
            tc.tile_pool(name="ps_convB", bufs=2, space="PSUM"))
        ps_dw = ctx.enter_context(
            tc.tile_pool(name="ps_dw", bufs=2, space="PSUM"))
        ps_tr = ctx.enter_context(
            tc.tile_pool(name="ps_tr", bufs=1, space="PSUM"))
        ps_sm = ctx.enter_context(
            tc.tile_pool(name="ps_sm", bufs=1, space="PSUM"))

        # -------- constants (host-prepped, few DMAs) --------
        # PE clock warm-up during the initial DMA fill: the HAM releases the
        # 1.2->2.4 GHz gate after ~3.4us of sustained activity.
        wsrc = consts.tile([128, 384], F32)
        nc.vector.memset(wsrc[:], 0.0)
        for _ in range(8):
            wps = ps_dw.tile([128, 256], F32, tag="dwps", name="warmps")
            nc.tensor.matmul(wps[:], wsrc[:, 0:128], wsrc[:, 128:384],
                             start=True, stop=True)

        biases = consts.tile([128, 3], F32)
        nc.scalar.dma_start(biases[:], d_biases)
        lhsA_t = consts.tile([128, 128], F32)
        nc.scalar.dma_start(lhsA_t[:], d_lhsA)
        lhsB_t = consts.tile([128, 128], BF16)
        nc.scalar.dma_start(lhsB_t[:], d_lhsB)
        dw_diag = consts.tile([128, 9 * 128], BF16)
        nc.scalar.dma_start(dw_diag[:], d_dwdiag)
        w9 = consts.tile([128, 9], F32)
        nc.scalar.dma_start(w9[:], d_dww9)
        lhsA, lhsB = lhsA_t[:], lhsB_t[:]
        biasA, biasB, dwb_t = (biases[:, 0:1], biases[:, 1:2],
                               biases[:, 2:3])

        # preload the ACT function tables (Gelu+Exp) off the critical path
        tdum = consts.tile([128, 1], F32)
        nc.scalar.activation(tdum[:], biases[:, 0:1], ACTF.Gelu,
                             bias=biases[:, 2:3])
        nc.scalar.activation(tdum[:], biases[:, 0:1], ACTF.Exp,
                             bias=biases[:, 2:3])

        id_f32 = consts.tile([128, 128], F32)
        make_identity(nc, id_f32[:])

        def one_pass():
            # v_pack (bf16): [v(b0) 0:32 | v(b1) 32:64]
            v_pack = packp.tile([C2, HW], BF16, tag="v_pack")
            # rt (f32): h-pooled [q(b0)|q(b1) sums 0:64 | k(b0)|k(b1) max 64:128]
            rt = packp.tile([128, H * (W // 2)], F32, tag="rt")
            # vertical-pooled: qf[0:32]=b0, [32:64]=b1; kf[64:96]=b0, [96:128]=b1
            qkpool = poolt.tile([128, NP], F32, tag="qkpool")
            qf = qkpool[0:C2, :]
            kf = qkpool
            # packed transposed pools: chunk j2 cols [j2*64, j2*64+64) =
            # [T(b0) 32 | T(b1) 32], bf16 for fast qk matmuls
            qfT = attnp.tile([128, (NP // 128) * C2], F32, tag="qfT")
            kfT = attnp.tile([128, (NP // 128) * C2], F32, tag="kfT")
            # packed qk accumulator [64,64]: diag blocks = per-batch qk[d,c];
            # off-diag blocks are cross-batch garbage (ignored). One group.
            qkt = ps_sm.tile([C2, C2], F32, tag="qkt")

            xin_tiles = {}
            xb_tiles = {}

            xs_tiles = {}

            def cast_x1_strip(s):
                xb = x1p.tile([128, (R + 2) * W], BF16, tag="xb",
                              name="xb", bufs=2)
                xb_tiles[s] = xb
                xin, rs, nrows = xin_tiles.pop(s)
                if s == 0:
                    nc.gpsimd.memset(xb[:, 0:W], 0.0)
                if s == NSTRIPS - 1:
                    nc.gpsimd.memset(xb[:, (R + 1) * W:(R + 2) * W], 0.0)
                nc.vector.tensor_copy(xb[:, rs * W:(rs + nrows) * W],
                                      xin[:, rs * W:(rs + nrows) * W])
                if s in DVE_STRIPS:
                    # padded-row shifted copy: xs3[:, r, 1+w] = xb[r, w] with
                    # zero columns 0 and 129: dx=+-1 taps read
                    # xs3[:, r, 1+dx : 129+dx] at even element offsets.
                    xs = x1p.tile([128, (R + 2) * WP], BF16, tag="xs",
                                  name="xs", bufs=1)
                    xs_tiles[s] = xs
                    xs3 = xs.rearrange("p (r w) -> p r w", w=WP)
                    xbr = xb.rearrange("p (r w) -> p r w", w=W)
                    nc.scalar.copy(xs3[:, :, 1:W + 1], xbr[:])
                    nc.gpsimd.memset(xs3[:, :, 0:1], 0.0)
                    nc.gpsimd.memset(xs3[:, :, WP - 1:WP], 0.0)

            def load_x1_strip(s):
                y0 = s * R
                ys = max(y0 - 1, 0)
                ye = min(y0 + R + 1, H)
                rs = 0 if s > 0 else 1
                nrows = ye - ys

                # contiguous (unpadded) strip rows: 1 descriptor per
                # partition per image -> cheap DGE + full DMA spread
                xin = x1p.tile([128, (R + 2) * W], F32, tag="xin",
                               name="xin", bufs=2)
                xin_tiles[s] = (xin, rs, nrows)
                nc.sync.dma_start(
                    xin[0:C2, rs * W:(rs + nrows) * W],
                    x[0, 0:C2, ys:ye, :].rearrange("c h w -> c (h w)"))
                nc.sync.dma_start(
                    xin[C2:128, rs * W:(rs + nrows) * W],
                    x[1, 0:C2, ys:ye, :].rearrange("c h w -> c (h w)"))

            ox_tiles = {}

            def compute_dw_chunk(s, q):
                # one 512-col (4-row) dwconv chunk of strip s: 9 accumulating
                # matmuls + gelu evac; DMA the strip out after its 4th chunk.
                xb = xb_tiles[s]
                xb3 = xb.rearrange("p (r w) -> p r w", w=W)
                if q == 0:
                    ox_tiles[s] = x1p.tile([128, R * W], F32, tag="ox1",
                                           name="ox1", bufs=2)
                ox1 = ox_tiles[s]
                if s in DVE_STRIPS:
                    xs3 = xs_tiles[s].rearrange("p (r w) -> p r w", w=WP)
                    acc = x1p.tile([128, NCH], BF16, tag="dacc", name="dacc",
                                   bufs=2)
                    for i, (dy, dx) in enumerate(TAPS):
                        r0 = 1 + q * RPC + dy
                        if dx == 0:
                            srcv = xb[:, r0 * W:r0 * W + NCH]
                        else:
                            srcv = xs3[:, r0:r0 + RPC, 1 + dx:1 + dx + W]
                        wcol = w9[:, i:i + 1]
                        if i == 0:
                            nc.vector.tensor_scalar_mul(acc[:], srcv, wcol)
                        else:
                            tmp = x1p.tile([128, NCH], BF16, tag="dtmp",
                                           name="dtmp", bufs=1)
                            nc.vector.tensor_scalar_mul(tmp[:], srcv, wcol)
                            nc.vector.tensor_add(acc[:], acc[:], tmp[:])
                    nc.scalar.activation(ox1[:, q * NCH:(q + 1) * NCH],
                                         acc[:], ACTF.Gelu,
                                         bias=dwb_t[:, 0:1])
                else:
                    ps = ps_dw.tile([128, NCH], F32, tag="dwps", name="dwps")
                    ps3 = ps.rearrange("p (r w) -> p r w", w=W)
                    order = sorted(range(9), key=lambda t: TAPS[t] != (0, 0))
                    for i, t in enumerate(order):
                        dy, dx = TAPS[t]
                        r0 = 1 + q * RPC + dy
                        lo, hi = max(0, -dx), W + min(0, -dx)
                        nc.tensor.matmul(
                            ps3[:, :, lo:hi],
                            dw_diag[:, t * 128:(t + 1) * 128],
                            xb3[:, r0:r0 + RPC, lo + dx:hi + dx],
                            start=(i == 0), stop=(i == 8))
                    nc.scalar.activation(ox1[:, q * NCH:(q + 1) * NCH],
                                         ps[:], ACTF.Gelu, bias=dwb_t[:, 0:1])
                if q == R // RPC - 1:
                    if s + 1 in xin_tiles:
                        cast_x1_strip(s + 1)
                    del xb_tiles[s]
                    del ox_tiles[s]
                    xs_tiles.pop(s, None)
                    rng = slice(s * R * W, (s + 1) * R * W)
                    nc.scalar.dma_start(oflat[0, 0:C2, rng], ox1[0:C2, :])
                    nc.scalar.dma_start(oflat[1, 0:C2, rng], ox1[C2:128, :])

            rhs_tiles = {}

            def load_rhs_quad(pj):
                rhs = rhsp.tile([128, 4 * NCH], F32, tag="rhs",
                                name="rhs", bufs=2)
                rhs_tiles[pj] = rhs
                cols4 = slice(4 * pj * NCH, (4 * pj + 4) * NCH)
                nc.sync.dma_start(rhs[0:C2, :], xflat[0, C2:C, cols4])
                nc.sync.dma_start(rhs[C2:128, :], xflat[1, C2:C, cols4])

            def emit_attn_quarter(qt):
                # vertical pool for output rows oy in [qt*16, (qt+1)*16)
                oy0, oy1 = qt * (R), (qt + 1) * R
                rq = rt[0:C2, :].rearrange(
                    "p (h2 two w2) -> p h2 two w2", two=2, w2=W // 2)
                qf3 = qf.rearrange("p (h2 w2) -> p h2 w2", w2=W // 2)
                nc.gpsimd.tensor_add(qf3[:, oy0:oy1, :],
                                     rq[:, oy0:oy1, 0, :],
                                     rq[:, oy0:oy1, 1, :])
                lo = max(oy0, 1)
                nc.gpsimd.tensor_add(qf3[:, lo:oy1, :], qf3[:, lo:oy1, :],
                                     rq[:, lo - 1:oy1 - 1, 1, :])
                rk = rt[C2:128, :].rearrange(
                    "p (h2 two w2) -> p h2 two w2", two=2, w2=W // 2)
                kf3 = kf[C2:128, :].rearrange("p (h2 w2) -> p h2 w2",
                                              w2=W // 2)
                nc.vector.tensor_max(kf3[:, oy0:oy1, :],
                                     rk[:, oy0:oy1, 0, :],
                                     rk[:, oy0:oy1, 1, :])

                # packed transposes ([64,128] -> [128,64]) + qk matmuls
                # for this quarter's 8 column chunks of 128
                for (src, base, dstT) in ((qf, 0, qfT), (kf, C2, kfT)):
                    ps = ps_tr.tile([128, 512], F32, tag="trps", name="trps")
                    for jj in range(8):
                        j2 = qt * 8 + jj
                        nc.tensor.transpose(
                            ps[:, jj * C2:(jj + 1) * C2],
                            src[base:base + C2, j2 * 128:(j2 + 1) * 128],
                            id_f32[base:base + C2, base:base + C2])
                    nc.vector.tensor_copy(
                        dstT[:, qt * 512:(qt + 1) * 512], ps[:])
                for jj in range(8):
                    j2 = qt * 8 + jj
                    sl = slice(j2 * C2, (j2 + 1) * C2)
                    nc.tensor.matmul(qkt[:], kfT[:, sl], qfT[:, sl],
                                     start=(qt == 0 and jj == 0),
                                     stop=(qt == 3 and jj == 7))

            # ---------- conv1x1 + dwconv chunks + attn, interleaved ----------
            load_x1_strip(0)
            load_rhs_quad(0)
            cast_x1_strip(0)
            vl = None
            for j in range(NCHUNKS):
                cols = bass.ts(j, NCH)
                if j % 4 == 0:
                    if j // 4 + 1 < NCHUNKS // 4:
                        load_rhs_quad(j // 4 + 1)
                    rhs = rhs_tiles.pop(j // 4)
                    rhsb = rhsp.tile([128, 4 * NCH], BF16, tag="rhsb",
                                     name="rhsb", bufs=2)
                    nc.vector.tensor_copy(rhsb[:], rhs[:])
                rhsv = rhs[:, (j % 4) * NCH:(j % 4 + 1) * NCH]
                rhsvb = rhsb[:, (j % 4) * NCH:(j % 4 + 1) * NCH]

                Ap = ps_convA.tile([128, NCH], F32, tag="convA", name="convA")
                nc.tensor.matmul(Ap[:], lhsA, rhsv, start=True, stop=True)
                qg = rhsp.tile([128, NCH], F32, tag="qg")
                nc.scalar.activation(qg[:], Ap[:], ACTF.Gelu,
                                     bias=biasA[:, 0:1])

                # fused horizontal pooling into rt (both batches per op)
                rrows = rt[:, j * RPC * (W // 2):(j + 1) * RPC * (W // 2)]
                Xq = qg[0:C2, :].rearrange(
                    "p (h w2 two) -> p h w2 two", h=RPC, two=2)
                r3 = rrows[0:C2, :].rearrange("p (h w2) -> p h w2", h=RPC)
                nc.gpsimd.tensor_add(r3[:], Xq[:, :, :, 0], Xq[:, :, :, 1])
                nc.gpsimd.tensor_add(r3[:, :, 1:W // 2], r3[:, :, 1:W // 2],
                                     Xq[:, :, 0:W // 2 - 1, 1])
                Xk = qg[C2:128, :].rearrange(
                    "p (h w2 two) -> p h w2 two", h=RPC, two=2)
                m3 = rrows[C2:128, :].rearrange("p (h w2) -> p h w2", h=RPC)
                nc.vector.tensor_max(m3[:], Xk[:, :, :, 0], Xk[:, :, :, 1])

                Bp = ps_convB.tile([128, NCH], F32, tag="convB", name="convB")
                nc.tensor.matmul(Bp[:], lhsB, rhsvb, start=True, stop=True)
                # single gelu for v (rows 0:64) + l (rows 64:128); DVE casts
                # the v half to bf16, the l half is DMAd out per chunk-pair
                if j % 2 == 0:
                    vl = rhsp.tile([128, 2 * NCH], F32, tag="vl", bufs=2)
                vls = vl[:, (j % 2) * NCH:(j % 2 + 1) * NCH]
                nc.scalar.activation(vls, Bp[:], ACTF.Gelu,
                                     bias=biasB[:, 0:1])
                nc.vector.tensor_copy(v_pack[:, cols], vls[0:C2, :])
                if j % 2 == 1:
                    cols2 = slice((j - 1) * NCH, (j + 1) * NCH)
                    nc.scalar.dma_start(oflat[0, C2:96, cols2],
                                        vl[C2:96, :])
                    nc.scalar.dma_start(oflat[1, C2:96, cols2],
                                        vl[96:128, :])

                s, q = j // 4, j % 4
                if q == 0 and s + 1 < NSTRIPS:
                    load_x1_strip(s + 1)
                compute_dw_chunk(s, q)
                if j in (8, 16, 24):
                    emit_attn_quarter(j // 8 - 1)

            # ---------- attention stats + packed out2 ----------
            emit_attn_quarter(3)
            qkts = attnp.tile([C2, C2], F32, tag="qkts")
            nc.vector.tensor_scalar_mul(qkts[:], qkt[:], 1.0 / 9.0)
            nmax = attnp.tile([C2, 1], F32, tag="nmax")
            ET = attnp.tile([C2, C2], F32, tag="ET")
            ssum = attnp.tile([C2, 1], F32, tag="ssum")
            rec = attnp.tile([C2, 1], F32, tag="rec")
            ETn = attnp.tile([C2, C2], F32, tag="ETn")
            nc.vector.memset(ETn[:], 0.0)
            for bi in range(BPC):
                rs = slice(C4 * bi, C4 * bi + C4)
                blk = (rs, slice(C4 * bi, C4 * bi + C4))
                nc.vector.tensor_reduce(nmax[rs, :], qkts[blk], axis=AX.X,
                                        op=ALU.max, negate=True)
                nc.scalar.activation(ET[blk], qkts[blk], ACTF.Exp,
                                     bias=nmax[rs, 0:1])
                nc.vector.reduce_sum(ssum[rs, :], ET[blk], axis=AX.X)
                nc.vector.reciprocal(rec[rs, :], ssum[rs, :])
                nc.vector.tensor_scalar_mul(ETn[blk], ET[blk], rec[rs, 0:1])
            etp = ps_sm.tile([C2, C2], F32, tag="qkt", name="etp")
            nc.tensor.transpose(etp[:], ETn[:], id_f32[0:C2, 0:C2])
            E_pack = attnp.tile([C2, C2], BF16, tag="E_pack")
            nc.vector.tensor_copy(E_pack[:], etp[:])

            # out2: one packed matmul per chunk -> [64,512] psum
            # (partitions p=(b*32+d)); DVE evac; one DMA per 2 chunks.
            st = None
            for j in range(NCHUNKS):
                cols = bass.ts(j, NCH)
                pool = ps_convA if j % 2 == 0 else ps_convB
                tag = "convA" if j % 2 == 0 else "convB"
                o2 = pool.tile([C2, NCH], F32, tag=tag, name="o2ps")
                nc.tensor.matmul(o2[:], E_pack[:], v_pack[:, cols],
                                 start=True, stop=True)
                if j % 2 == 0:
                    st = stgp.tile([C2, 2 * NCH], F32, tag="st", bufs=3)
                dst = st[:, (j % 2) * NCH:(j % 2 + 1) * NCH]
                if j % 2 == 0:
                    nc.vector.tensor_copy(dst, o2[:])
                else:
                    nc.scalar.copy(dst, o2[:])
                if j % 2 == 1:
                    cols2 = slice((j - 1) * NCH, (j + 1) * NCH)
                    nc.gpsimd.dma_start(oflat[0, 96:128, cols2],
                                        st[0:C4, :])
                    nc.gpsimd.dma_start(oflat[1, 96:128, cols2],
                                        st[C4:C2, :])

        for _ in range(loops):
            one_pass()

    nc.compile()
    return nc


def prep_params(dw_w, dw_b, qkvl_w, qkvl_b):
    """Host-side packing of the small parameter tensors into the layouts the
    kernel consumes (block-diagonal conv weights, batch-replicated biases,
    9 diagonal dwconv blocks)."""
    f = np.float32
    qwT = np.ascontiguousarray(
        np.asarray(qkvl_w, f).reshape(C, C2).T)          # [64ic, 128oc]
    lhsA = np.zeros((128, 128), f)
    lhsB = np.zeros((128, 128), f)
    lhsA[0:C2, 0:C4] = qwT[:, 0:C4]
    lhsA[C2:128, C4:C2] = qwT[:, 0:C4]
    lhsA[0:C2, C2:96] = qwT[:, C4:C2]
    lhsA[C2:128, 96:128] = qwT[:, C4:C2]
    lhsB[0:C2, 0:C4] = qwT[:, C2:96]
    lhsB[C2:128, C4:C2] = qwT[:, C2:96]
    lhsB[0:C2, C2:96] = qwT[:, 96:128]
    lhsB[C2:128, 96:128] = qwT[:, 96:128]
    qb = np.asarray(qkvl_b, f)
    biasA = np.concatenate([qb[0:C4], qb[0:C4], qb[C4:C2], qb[C4:C2]])
    biasB = np.concatenate([qb[C2:96], qb[C2:96], qb[96:128], qb[96:128]])
    dwb = np.tile(np.asarray(dw_b, f), 2)
    w9 = np.asarray(dw_w, f).reshape(C2, 9)
    wfull = np.vstack([w9, w9])                          # [128, 9]
    dwdiag = np.zeros((128, 9, 128), f)
    for t in range(9):
        dwdiag[np.arange(128), t, np.arange(128)] = wfull[:, t]
    bf = ml_dtypes.bfloat16
    return {
        "lhsA": np.ascontiguousarray(lhsA),
        "lhsB": np.ascontiguousarray(lhsB.astype(bf)),
        "biases": np.ascontiguousarray(
            np.stack([biasA, biasB, dwb], axis=1)),
        "dwdiag": np.ascontiguousarray(
            dwdiag.reshape(128, 9 * 128).astype(bf)),
        "dww9": np.ascontiguousarray(wfull),
    }


_NC_CACHE = None


def _get_nc():
    global _NC_CACHE
    if _NC_CACHE is None:
        _NC_CACHE = build_nc()
    return _NC_CACHE


def kernel(x, dw_w, dw_b, qkvl_w, qkvl_b):
    x = np.ascontiguousarray(np.asarray(x, dtype=np.float32))
    shared = prep_params(dw_w, dw_b, qkvl_w, qkvl_b)
    nc = _get_nc()
    in_maps = [
        {"x": x[c * BPC:(c + 1) * BPC], **shared} for c in range(N_CORES)
    ]
    res = bass_utils.run_bass_kernel_spmd(nc, in_maps,
                                          core_ids=list(range(N_CORES)))
    return np.concatenate([res.results[c]["out"] for c in range(N_CORES)],
                          axis=0)


# revision 26
# speedup vs baseline: 1.0309x; 1.0309x over previous
"""Trainium2 Bass kernel for the ELGCA block (dwconv3x3+gelu || conv1x1+gelu
-> pooled linear attention), data-parallel over batch on 8 NeuronCores.

Self-contained: hardcodes shapes B=16, C=128, H=W=128, f32.
kernel(**inputs) takes full unsharded inputs, returns the FULL output.

Per-core layout (BPC=2 local images b0,b1), partitions p=(b*64+c):
  dwconv3x3: 9 accumulating PE matmuls per 512-col chunk; lhsT = diagonal
             [128x128] of tap weights (fp32r), rhs = flat-shifted window of
             the zero-padded x1 strip (fp32r).  PSUM holds the 9-tap sum in
             f32; ACT applies bias+gelu on evacuation.
  conv1x1:   block-diagonal fp32r matmuls per 512-col chunk;
             A psum = [q(b0)|q(b1)|k(b0)|k(b1)], B = [v(b0)|v(b1)|l(b0)|l(b1)].
  attention: pooled qf/kf -> per-quarter packed transposes -> qk accumulated
             in one [32,64] PSUM bank -> softmax stats -> packed out2 matmul.
"""

import ml_dtypes
import numpy as np
from contextlib import ExitStack

import concourse.bass as bass
import concourse.tile as tile
from concourse import bacc, mybir
from concourse import bass_utils
from concourse.masks import make_identity

F32 = mybir.dt.float32
F32R = mybir.dt.float32r
BF16 = mybir.dt.bfloat16
AX = mybir.AxisListType
ALU = mybir.AluOpType
ACTF = mybir.ActivationFunctionType

N_CORES = 8
B_TOT, C, H, W = 16, 128, 128, 128
BPC = B_TOT // N_CORES          # 2 images per core
HW = H * W                      # 16384
C2 = C // 2                     # 64
C4 = C // 4                     # 32
WP = W + 2                      # padded row width for dwconv
R = 16                          # dwconv row-strip height
NCH = 512                       # conv1x1 / out2 column chunk
NCHUNKS = HW // NCH             # 32
NSTRIPS = H // R                # 8
RPC = NCH // W                  # image rows per 512-col chunk (4)
NP = (H // 2) * (W // 2)        # 4096 pooled positions

DVE_STRIPS = ()

# dwconv taps in row-major (dy, dx) order
TAPS = [(dy, dx) for dy in (-1, 0, 1) for dx in (-1, 0, 1)]


def build_nc(loops=1):
    nc = bacc.Bacc("TRN2", target_bir_lowering=False, debug=False,
                   num_devices=N_CORES)
    x = nc.dram_tensor("x", [BPC, C, H, W], F32, kind="ExternalInput").ap()
    d_lhsA = nc.dram_tensor("lhsA", [128, 128], F32,
                            kind="ExternalInput").ap()
    d_lhsB = nc.dram_tensor("lhsB", [128, 128], BF16,
                            kind="ExternalInput").ap()
    d_biases = nc.dram_tensor("biases", [128, 3], F32,
                              kind="ExternalInput").ap()
    d_dwdiag = nc.dram_tensor("dwdiag", [128, 9 * 128], BF16,
                              kind="ExternalInput").ap()
    d_dww9 = nc.dram_tensor("dww9", [128, 9], F32,
                            kind="ExternalInput").ap()
    out = nc.dram_tensor("out", [BPC, C, H, W], F32, kind="ExternalOutput").ap()

    xflat = x.rearrange("b c h w -> b c (h w)")
    oflat = out.rearrange("b c h w -> b c (h w)")

    with tile.TileContext(nc) as tc, ExitStack() as ctx:
        consts = ctx.enter_context(tc.tile_pool(name="consts", bufs=1))
        x1p = ctx.enter_context(tc.tile_pool(name="x1p", bufs=2))
        rhsp = ctx.enter_context(tc.tile_pool(name="rhsp", bufs=2))
        packp = ctx.enter_context(tc.tile_pool(name="packp", bufs=1))
        poolt = ctx.enter_context(tc.tile_pool(name="poolt", bufs=1))
        attnp = ctx.enter_context(tc.tile_pool(name="attnp", bufs=1))
        stgp = ctx.enter_context(tc.tile_pool(name="stgp", bufs=1))
        ps_convA = ctx.enter_context(
            tc.tile_pool(name="ps_convA", bufs=2, space="PSUM"))
        ps_convB = ctx.enter_context(
            tc.tile_pool(name="ps_convB", bufs=2, space="PSUM"))
        ps_dw = ctx.enter_context(
            tc.tile_pool(name="ps_dw", bufs=2, space="PSUM"))
        ps_tr = ctx.enter_context(
            tc.tile_pool(name="ps_tr", bufs=1, space="PSUM"))
        ps_sm = ctx.enter_context(
            tc.tile_pool(name="ps_sm", bufs=1, space="PSUM"))

        # -------- constants (host-prepped, few DMAs) --------
        biases = consts.tile([128, 3], F32)
        nc.scalar.dma_start(biases[:], d_biases)
        lhsA_t = consts.tile([128, 128], F32)
        nc.scalar.dma_start(lhsA_t[:], d_lhsA)
        lhsB_t = consts.tile([128, 128], BF16)
        nc.scalar.dma_start(lhsB_t[:], d_lhsB)
        dw_diag = consts.tile([128, 9 * 128], BF16)
        nc.scalar.dma_start(dw_diag[:], d_dwdiag)
        w9 = consts.tile([128, 9], F32)
        nc.scalar.dma_start(w9[:], d_dww9)
        lhsA, lhsB = lhsA_t[:], lhsB_t[:]
        biasA, biasB, dwb_t = (biases[:, 0:1], biases[:, 1:2],
                               biases[:, 2:3])

        # preload the ACT function tables (Gelu+Exp) off the critical path
        tdum = consts.tile([128, 1], F32)
        nc.scalar.activation(tdum[:], biases[:, 0:1], ACTF.Gelu,
                             bias=biases[:, 2:3])
        nc.scalar.activation(tdum[:], biases[:, 0:1], ACTF.Exp,
                             bias=biases[:, 2:3])

        id_f32 = consts.tile([128, 128], F32)
        make_identity(nc, id_f32[:])

        # PE clock warm-up during the initial DMA fill: the HAM releases the
        # 1.2->2.4 GHz gate after ~3.4us of sustained activity.
        for _ in range(10):
            wps = ps_dw.tile([128, 128], F32, tag="dwps", name="warmps")
            nc.tensor.matmul(wps[:], id_f32[:], id_f32[:],
                             start=True, stop=True)

        def one_pass():
            # v_pack (bf16): [v(b0) 0:32 | v(b1) 32:64]
            v_pack = packp.tile([C2, HW], BF16, tag="v_pack")
            # rt (f32): h-pooled [q(b0)|q(b1) sums 0:64 | k(b0)|k(b1) max 64:128]
            rt = packp.tile([128, H * (W // 2)], F32, tag="rt")
            # vertical-pooled: qf[0:32]=b0, [32:64]=b1; kf[64:96]=b0, [96:128]=b1
            qkpool = poolt.tile([128, NP], F32, tag="qkpool")
            qf = qkpool[0:C2, :]
            kf = qkpool
            # packed transposed pools: chunk j2 cols [j2*64, j2*64+64) =
            # [T(b0) 32 | T(b1) 32], bf16 for fast qk matmuls
            qfT = attnp.tile([128, (NP // 128) * C2], F32, tag="qfT")
            kfT = attnp.tile([128, (NP // 128) * C2], F32, tag="kfT")
            # packed qk accumulator [64,64]: diag blocks = per-batch qk[d,c];
            # off-diag blocks are cross-batch garbage (ignored). One group.
            qkt = ps_sm.tile([C2, C2], F32, tag="qkt")

            xin_tiles = {}
            xb_tiles = {}

            xs_tiles = {}

            def cast_x1_strip(s):
                xb = x1p.tile([128, (R + 2) * W], BF16, tag="xb",
                              name="xb", bufs=3)
                xb_tiles[s] = xb
                xin, rs, nrows = xin_tiles.pop(s)
                if s == 0:
                    nc.gpsimd.memset(xb[:, 0:W], 0.0)
                if s == NSTRIPS - 1:
                    nc.gpsimd.memset(xb[:, (R + 1) * W:(R + 2) * W], 0.0)
                nc.vector.tensor_copy(xb[:, rs * W:(rs + nrows) * W],
                                      xin[:, rs * W:(rs + nrows) * W])
                if s in DVE_STRIPS:
                    # padded-row shifted copy: xs3[:, r, 1+w] = xb[r, w] with
                    # zero columns 0 and 129: dx=+-1 taps read
                    # xs3[:, r, 1+dx : 129+dx] at even element offsets.
                    xs = x1p.tile([128, (R + 2) * WP], BF16, tag="xs",
                                  name="xs", bufs=1)
                    xs_tiles[s] = xs
                    xs3 = xs.rearrange("p (r w) -> p r w", w=WP)
                    xbr = xb.rearrange("p (r w) -> p r w", w=W)
                    nc.scalar.copy(xs3[:, :, 1:W + 1], xbr[:])
                    nc.gpsimd.memset(xs3[:, :, 0:1], 0.0)
                    nc.gpsimd.memset(xs3[:, :, WP - 1:WP], 0.0)

            def load_x1_strip(s):
                y0 = s * R
                ys = max(y0 - 1, 0)
                ye = min(y0 + R + 1, H)
                rs = 0 if s > 0 else 1
                nrows = ye - ys

                # contiguous (unpadded) strip rows: 1 descriptor per
                # partition per image -> cheap DGE + full DMA spread
                xin = x1p.tile([128, (R + 2) * W], F32, tag="xin",
                               name="xin", bufs=2)
                xin_tiles[s] = (xin, rs, nrows)
                nc.sync.dma_start(
                    xin[0:C2, rs * W:(rs + nrows) * W],
                    x[0, 0:C2, ys:ye, :].rearrange("c h w -> c (h w)"))
                nc.sync.dma_start(
                    xin[C2:128, rs * W:(rs + nrows) * W],
                    x[1, 0:C2, ys:ye, :].rearrange("c h w -> c (h w)"))

            ox_tiles = {}

            def compute_dw_chunk(s, q):
                # one 512-col (4-row) dwconv chunk of strip s: 9 accumulating
                # matmuls + gelu evac; DMA the strip out after its 4th chunk.
                xb = xb_tiles[s]
                xb3 = xb.rearrange("p (r w) -> p r w", w=W)
                if q == 0:
                    ox_tiles[s] = x1p.tile([128, R * W], F32, tag="ox1",
                                           name="ox1", bufs=2)
                ox1 = ox_tiles[s]
                if s in DVE_STRIPS:
                    xs3 = xs_tiles[s].rearrange("p (r w) -> p r w", w=WP)
                    acc = x1p.tile([128, NCH], BF16, tag="dacc", name="dacc",
                                   bufs=2)
                    for i, (dy, dx) in enumerate(TAPS):
                        r0 = 1 + q * RPC + dy
                        if dx == 0:
                            srcv = xb[:, r0 * W:r0 * W + NCH]
                        else:
                            srcv = xs3[:, r0:r0 + RPC, 1 + dx:1 + dx + W]
                        wcol = w9[:, i:i + 1]
                        if i == 0:
                            nc.vector.tensor_scalar_mul(acc[:], srcv, wcol)
                        else:
                            tmp = x1p.tile([128, NCH], BF16, tag="dtmp",
                                           name="dtmp", bufs=1)
                            nc.vector.tensor_scalar_mul(tmp[:], srcv, wcol)
                            nc.vector.tensor_add(acc[:], acc[:], tmp[:])
                    nc.scalar.activation(ox1[:, q * NCH:(q + 1) * NCH],
                                         acc[:], ACTF.Gelu,
                                         bias=dwb_t[:, 0:1])
                else:
                    ps = ps_dw.tile([128, NCH], F32, tag="dwps", name="dwps")
                    ps3 = ps.rearrange("p (r w) -> p r w", w=W)
                    order = sorted(range(9), key=lambda t: TAPS[t] != (0, 0))
                    for i, t in enumerate(order):
                        dy, dx = TAPS[t]
                        r0 = 1 + q * RPC + dy
                        lo, hi = max(0, -dx), W + min(0, -dx)
                        nc.tensor.matmul(
                            ps3[:, :, lo:hi],
                            dw_diag[:, t * 128:(t + 1) * 128],
                            xb3[:, r0:r0 + RPC, lo + dx:hi + dx],
                            start=(i == 0), stop=(i == 8))
                    nc.scalar.activation(ox1[:, q * NCH:(q + 1) * NCH],
                                         ps[:], ACTF.Gelu, bias=dwb_t[:, 0:1])
                if q == R // RPC - 1:
                    if s + 1 in xin_tiles:
                        cast_x1_strip(s + 1)
                    del xb_tiles[s]
                    del ox_tiles[s]
                    xs_tiles.pop(s, None)
                    rng = slice(s * R * W, (s + 1) * R * W)
                    nc.scalar.dma_start(oflat[0, 0:C2, rng], ox1[0:C2, :])
                    nc.scalar.dma_start(oflat[1, 0:C2, rng], ox1[C2:128, :])

            rhs_tiles = {}

            def load_rhs_pair(pj):
                rhs = rhsp.tile([128, 2 * NCH], F32, tag="rhs",
                                name="rhs", bufs=3)
                rhs_tiles[pj] = rhs
                cols2 = slice(2 * pj * NCH, (2 * pj + 2) * NCH)
                nc.sync.dma_start(rhs[0:C2, :], xflat[0, C2:C, cols2])
                nc.sync.dma_start(rhs[C2:128, :], xflat[1, C2:C, cols2])

            def emit_attn_quarter(qt):
                # vertical pool for output rows oy in [qt*16, (qt+1)*16)
                oy0, oy1 = qt * (R), (qt + 1) * R
                rq = rt[0:C2, :].rearrange(
                    "p (h2 two w2) -> p h2 two w2", two=2, w2=W // 2)
                qf3 = qf.rearrange("p (h2 w2) -> p h2 w2", w2=W // 2)
                nc.gpsimd.tensor_add(qf3[:, oy0:oy1, :],
                                     rq[:, oy0:oy1, 0, :],
                                     rq[:, oy0:oy1, 1, :])
                lo = max(oy0, 1)
                nc.gpsimd.tensor_add(qf3[:, lo:oy1, :], qf3[:, lo:oy1, :],
                                     rq[:, lo - 1:oy1 - 1, 1, :])
                rk = rt[C2:128, :].rearrange(
                    "p (h2 two w2) -> p h2 two w2", two=2, w2=W // 2)
                kf3 = kf[C2:128, :].rearrange("p (h2 w2) -> p h2 w2",
                                              w2=W // 2)
                nc.vector.tensor_max(kf3[:, oy0:oy1, :],
                                     rk[:, oy0:oy1, 0, :],
                                     rk[:, oy0:oy1, 1, :])

                # packed transposes ([64,128] -> [128,64]) + qk matmuls
                # for this quarter's 8 column chunks of 128
                for (src, base, dstT) in ((qf, 0, qfT), (kf, C2, kfT)):
                    ps = ps_tr.tile([128, 512], F32, tag="trps", name="trps")
                    for jj in range(8):
                        j2 = qt * 8 + jj
                        nc.tensor.transpose(
                            ps[:, jj * C2:(jj + 1) * C2],
                            src[base:base + C2, j2 * 128:(j2 + 1) * 128],
                            id_f32[base:base + C2, base:base + C2])
                    nc.vector.tensor_copy(
                        dstT[:, qt * 512:(qt + 1) * 512], ps[:])
                for jj in range(8):
                    j2 = qt * 8 + jj
                    sl = slice(j2 * C2, (j2 + 1) * C2)
                    nc.tensor.matmul(qkt[:], kfT[:, sl], qfT[:, sl],
                                     start=(qt == 0 and jj == 0),
                                     stop=(qt == 3 and jj == 7))

            # ---------- conv1x1 + dwconv chunks + attn, interleaved ----------
            load_x1_strip(0)
            load_rhs_pair(0)
            load_rhs_pair(1)
            cast_x1_strip(0)
            vl = None
            for j in range(NCHUNKS):
                cols = bass.ts(j, NCH)
                if j % 2 == 0:
                    if j // 2 + 2 < NCHUNKS // 2:
                        load_rhs_pair(j // 2 + 2)
                    rhs = rhs_tiles.pop(j // 2)
                    rhsb = rhsp.tile([128, 2 * NCH], BF16, tag="rhsb",
                                     name="rhsb", bufs=2)
                    nc.vector.tensor_copy(rhsb[:], rhs[:])
                rhsv = rhs[:, (j % 2) * NCH:(j % 2 + 1) * NCH]
                rhsvb = rhsb[:, (j % 2) * NCH:(j % 2 + 1) * NCH]

                Ap = ps_convA.tile([128, NCH], F32, tag="convA", name="convA")
                nc.tensor.matmul(Ap[:], lhsA, rhsv, start=True, stop=True)
                qg = rhsp.tile([128, NCH], F32, tag="qg")
                nc.scalar.activation(qg[:], Ap[:], ACTF.Gelu,
                                     bias=biasA[:, 0:1])

                # fused horizontal pooling into rt (both batches per op)
                rrows = rt[:, j * RPC * (W // 2):(j + 1) * RPC * (W // 2)]
                Xq = qg[0:C2, :].rearrange(
                    "p (h w2 two) -> p h w2 two", h=RPC, two=2)
                r3 = rrows[0:C2, :].rearrange("p (h w2) -> p h w2", h=RPC)
                nc.gpsimd.tensor_add(r3[:], Xq[:, :, :, 0], Xq[:, :, :, 1])
                nc.gpsimd.tensor_add(r3[:, :, 1:W // 2], r3[:, :, 1:W // 2],
                                     Xq[:, :, 0:W // 2 - 1, 1])
                Xk = qg[C2:128, :].rearrange(
                    "p (h w2 two) -> p h w2 two", h=RPC, two=2)
                m3 = rrows[C2:128, :].rearrange("p (h w2) -> p h w2", h=RPC)
                nc.vector.tensor_max(m3[:], Xk[:, :, :, 0], Xk[:, :, :, 1])

                Bp = ps_convB.tile([128, NCH], F32, tag="convB", name="convB")
                nc.tensor.matmul(Bp[:], lhsB, rhsvb, start=True, stop=True)
                # single gelu for v (rows 0:64) + l (rows 64:128); DVE casts
                # the v half to bf16, the l half is DMAd out per chunk-pair
                if j % 2 == 0:
                    vl = rhsp.tile([128, 2 * NCH], F32, tag="vl", bufs=2)
                vls = vl[:, (j % 2) * NCH:(j % 2 + 1) * NCH]
                nc.scalar.activation(vls, Bp[:], ACTF.Gelu,
                                     bias=biasB[:, 0:1])
                nc.vector.tensor_copy(v_pack[:, cols], vls[0:C2, :])
                if j % 2 == 1:
                    cols2 = slice((j - 1) * NCH, (j + 1) * NCH)
                    nc.scalar.dma_start(oflat[0, C2:96, cols2],
                                        vl[C2:96, :])
                    nc.scalar.dma_start(oflat[1, C2:96, cols2],
                                        vl[96:128, :])

                s, q = j // 4, j % 4
                if q == 0 and s + 1 < NSTRIPS:
                    load_x1_strip(s + 1)
                compute_dw_chunk(s, q)
                if j in (8, 16, 24):
                    emit_attn_quarter(j // 8 - 1)

            # ---------- attention stats + packed out2 ----------
            emit_attn_quarter(3)
            qkts = attnp.tile([C2, C2], F32, tag="qkts")
            nc.vector.tensor_scalar_mul(qkts[:], qkt[:], 1.0 / 9.0)
            nmax = attnp.tile([C2, 1], F32, tag="nmax")
            ET = attnp.tile([C2, C2], F32, tag="ET")
            ssum = attnp.tile([C2, 1], F32, tag="ssum")
            rec = attnp.tile([C2, 1], F32, tag="rec")
            ETn = attnp.tile([C2, C2], F32, tag="ETn")
            nc.vector.memset(ETn[:], 0.0)
            for bi in range(BPC):
                rs = slice(C4 * bi, C4 * bi + C4)
                blk = (rs, slice(C4 * bi, C4 * bi + C4))
                nc.vector.tensor_reduce(nmax[rs, :], qkts[blk], axis=AX.X,
                                        op=ALU.max, negate=True)
                nc.scalar.activation(ET[blk], qkts[blk], ACTF.Exp,
                                     bias=nmax[rs, 0:1])
                nc.vector.reduce_sum(ssum[rs, :], ET[blk], axis=AX.X)
                nc.vector.reciprocal(rec[rs, :], ssum[rs, :])
                nc.vector.tensor_scalar_mul(ETn[blk], ET[blk], rec[rs, 0:1])
            etp = ps_sm.tile([C2, C2], F32, tag="qkt", name="etp")
            nc.tensor.transpose(etp[:], ETn[:], id_f32[0:C2, 0:C2])
            E_pack = attnp.tile([C2, C2], BF16, tag="E_pack")
            nc.vector.tensor_copy(E_pack[:], etp[:])

            # out2: one packed matmul per chunk -> [64,512] psum
            # (partitions p=(b*32+d)); DVE evac; one DMA per 2 chunks.
            st = None
            for j in range(NCHUNKS):
                cols = bass.ts(j, NCH)
                pool = ps_convA if j % 2 == 0 else ps_convB
                tag = "convA" if j % 2 == 0 else "convB"
                o2 = pool.tile([C2, NCH], F32, tag=tag, name="o2ps")
                nc.tensor.matmul(o2[:], E_pack[:], v_pack[:, cols],
                                 start=True, stop=True)
                if j % 2 == 0:
                    st = stgp.tile([C2, 2 * NCH], F32, tag="st", bufs=3)
                dst = st[:, (j % 2) * NCH:(j % 2 + 1) * NCH]
                if j % 2 == 0:
                    nc.vector.tensor_copy(dst, o2[:])
                else:
                    nc.scalar.copy(dst, o2[:])
                if j % 2 == 1:
                    cols2 = slice((j - 1) * NCH, (j + 1) * NCH)
                    nc.gpsimd.dma_start(oflat[0, 96:128, cols2],
                                        st[0:C4, :])
                    nc.gpsimd.dma_start(oflat[1, 96:128, cols2],
                                        st[C4:C2, :])

        for _ in range(loops):
            one_pass()

    nc.compile()
    return nc


def prep_params(dw_w, dw_b, qkvl_w, qkvl_b):
    """Host-side packing of the small parameter tensors into the layouts the
    kernel consumes (block-diagonal conv weights, batch-replicated biases,
    9 diagonal dwconv blocks)."""
    f = np.float32
    qwT = np.ascontiguousarray(
        np.asarray(qkvl_w, f).reshape(C, C2).T)          # [64ic, 128oc]
    lhsA = np.zeros((128, 128), f)
    lhsB = np.zeros((128, 128), f)
    lhsA[0:C2, 0:C4] = qwT[:, 0:C4]
    lhsA[C2:128, C4:C2] = qwT[:, 0:C4]
    lhsA[0:C2, C2:96] = qwT[:, C4:C2]
    lhsA[C2:128, 96:128] = qwT[:, C4:C2]
    lhsB[0:C2, 0:C4] = qwT[:, C2:96]
    lhsB[C2:128, C4:C2] = qwT[:, C2:96]
    lhsB[0:C2, C2:96] = qwT[:, 96:128]
    lhsB[C2:128, 96:128] = qwT[:, 96:128]
    qb = np.asarray(qkvl_b, f)
    biasA = np.concatenate([qb[0:C4], qb[0:C4], qb[C4:C2], qb[C4:C2]])
    biasB = np.concatenate([qb[C2:96], qb[C2:96], qb[96:128], qb[96:128]])
    dwb = np.tile(np.asarray(dw_b, f), 2)
    w9 = np.asarray(dw_w, f).reshape(C2, 9)
    wfull = np.vstack([w9, w9])                          # [128, 9]
    dwdiag = np.zeros((128, 9, 128), f)
    for t in range(9):
        dwdiag[np.arange(128), t, np.arange(128)] = wfull[:, t]
    bf = ml_dtypes.bfloat16
    return {
        "lhsA": np.ascontiguousarray(lhsA),
        "lhsB": np.ascontiguousarray(lhsB.astype(bf)),
        "biases": np.ascontiguousarray(
            np.stack([biasA, biasB, dwb], axis=1)),
        "dwdiag": np.ascontiguousarray(
            dwdiag.reshape(128, 9 * 128).astype(bf)),
        "dww9": np.ascontiguousarray(wfull),
    }


_NC_CACHE = None


def _get_nc():
    global _NC_CACHE
    if _NC_CACHE is None:
        _NC_CACHE = build_nc()
    return _NC_CACHE


def kernel(x, dw_w, dw_b, qkvl_w, qkvl_b):
    x = np.ascontiguousarray(np.asarray(x, dtype=np.float32))
    shared = prep_params(dw_w, dw_b, qkvl_w, qkvl_b)
    nc = _get_nc()
    in_maps = [
        {"x": x[c * BPC:(c + 1) * BPC], **shared} for c in range(N_CORES)
    ]
    res = bass_utils.run_bass_kernel_spmd(nc, in_maps,
                                          core_ids=list(range(N_CORES)))
    return np.concatenate([res.results[c]["out"] for c in range(N_CORES)],
                          axis=0)


# revision 27
# speedup vs baseline: 1.0349x; 1.0038x over previous
"""Trainium2 Bass kernel for the ELGCA block (dwconv3x3+gelu || conv1x1+gelu
-> pooled linear attention), data-parallel over batch on 8 NeuronCores.

Self-contained: hardcodes shapes B=16, C=128, H=W=128, f32.
kernel(**inputs) takes full unsharded inputs, returns the FULL output.

Per-core layout (BPC=2 local images b0,b1), partitions p=(b*64+c):
  dwconv3x3: 9 accumulating PE matmuls per 512-col chunk; lhsT = diagonal
             [128x128] of tap weights (fp32r), rhs = flat-shifted window of
             the zero-padded x1 strip (fp32r).  PSUM holds the 9-tap sum in
             f32; ACT applies bias+gelu on evacuation.
  conv1x1:   block-diagonal fp32r matmuls per 512-col chunk;
             A psum = [q(b0)|q(b1)|k(b0)|k(b1)], B = [v(b0)|v(b1)|l(b0)|l(b1)].
  attention: pooled qf/kf -> per-quarter packed transposes -> qk accumulated
             in one [32,64] PSUM bank -> softmax stats -> packed out2 matmul.
"""

import ml_dtypes
import numpy as np
from contextlib import ExitStack

import concourse.bass as bass
import concourse.tile as tile
from concourse import bacc, mybir
from concourse import bass_utils
from concourse.masks import make_identity

F32 = mybir.dt.float32
F32R = mybir.dt.float32r
BF16 = mybir.dt.bfloat16
AX = mybir.AxisListType
ALU = mybir.AluOpType
ACTF = mybir.ActivationFunctionType

N_CORES = 8
B_TOT, C, H, W = 16, 128, 128, 128
BPC = B_TOT // N_CORES          # 2 images per core
HW = H * W                      # 16384
C2 = C // 2                     # 64
C4 = C // 4                     # 32
WP = W + 2                      # padded row width for dwconv
R = 16                          # dwconv row-strip height
NCH = 512                       # conv1x1 / out2 column chunk
NCHUNKS = HW // NCH             # 32
NSTRIPS = H // R                # 8
RPC = NCH // W                  # image rows per 512-col chunk (4)
NP = (H // 2) * (W // 2)        # 4096 pooled positions

DVE_STRIPS = ()

# dwconv taps in row-major (dy, dx) order
TAPS = [(dy, dx) for dy in (-1, 0, 1) for dx in (-1, 0, 1)]


def build_nc(loops=1):
    nc = bacc.Bacc("TRN2", target_bir_lowering=False, debug=False,
                   num_devices=N_CORES)
    x = nc.dram_tensor("x", [BPC, C, H, W], F32, kind="ExternalInput").ap()
    d_lhsA = nc.dram_tensor("lhsA", [128, 128], F32,
                            kind="ExternalInput").ap()
    d_lhsB = nc.dram_tensor("lhsB", [128, 128], BF16,
                            kind="ExternalInput").ap()
    d_biases = nc.dram_tensor("biases", [128, 3], F32,
                              kind="ExternalInput").ap()
    d_dwdiag = nc.dram_tensor("dwdiag", [128, 9 * 128], BF16,
                              kind="ExternalInput").ap()
    d_dww9 = nc.dram_tensor("dww9", [128, 9], F32,
                            kind="ExternalInput").ap()
    out = nc.dram_tensor("out", [BPC, C, H, W], F32, kind="ExternalOutput").ap()

    xflat = x.rearrange("b c h w -> b c (h w)")
    oflat = out.rearrange("b c h w -> b c (h w)")

    with tile.TileContext(nc) as tc, ExitStack() as ctx:
        consts = ctx.enter_context(tc.tile_pool(name="consts", bufs=1))
        x1p = ctx.enter_context(tc.tile_pool(name="x1p", bufs=2))
        rhsp = ctx.enter_context(tc.tile_pool(name="rhsp", bufs=2))
        packp = ctx.enter_context(tc.tile_pool(name="packp", bufs=1))
        poolt = ctx.enter_context(tc.tile_pool(name="poolt", bufs=1))
        attnp = ctx.enter_context(tc.tile_pool(name="attnp", bufs=1))
        stgp = ctx.enter_context(tc.tile_pool(name="stgp", bufs=1))
        ps_convA = ctx.enter_context(
            tc.tile_pool(name="ps_convA", bufs=2, space="PSUM"))
        ps_convB = ctx.enter_context(
            tc.tile_pool(name="ps_convB", bufs=2, space="PSUM"))
        ps_dw = ctx.enter_context(
            tc.tile_pool(name="ps_dw", bufs=2, space="PSUM"))
        ps_tr = ctx.enter_context(
            tc.tile_pool(name="ps_tr", bufs=1, space="PSUM"))
        ps_sm = ctx.enter_context(
            tc.tile_pool(name="ps_sm", bufs=1, space="PSUM"))

        # -------- constants (host-prepped, few DMAs) --------
        biases = consts.tile([128, 3], F32)
        nc.scalar.dma_start(biases[:], d_biases)
        lhsA_t = consts.tile([128, 128], F32)
        nc.scalar.dma_start(lhsA_t[:], d_lhsA)
        lhsB_t = consts.tile([128, 128], BF16)
        nc.scalar.dma_start(lhsB_t[:], d_lhsB)
        dw_diag = consts.tile([128, 9 * 128], BF16)
        nc.scalar.dma_start(dw_diag[:], d_dwdiag)
        w9 = consts.tile([128, 9], F32)
        nc.scalar.dma_start(w9[:], d_dww9)
        lhsA, lhsB = lhsA_t[:], lhsB_t[:]
        biasA, biasB, dwb_t = (biases[:, 0:1], biases[:, 1:2],
                               biases[:, 2:3])

        # preload the ACT function tables (Gelu+Exp) off the critical path
        tdum = consts.tile([128, 1], F32)
        nc.scalar.activation(tdum[:], biases[:, 0:1], ACTF.Gelu,
                             bias=biases[:, 2:3])
        nc.scalar.activation(tdum[:], biases[:, 0:1], ACTF.Exp,
                             bias=biases[:, 2:3])

        id_f32 = consts.tile([128, 128], F32)
        make_identity(nc, id_f32[:])

        # PE clock warm-up during the initial DMA fill: the HAM releases the
        # 1.2->2.4 GHz gate after ~3.4us of sustained activity.
        for _ in range(10):
            wps = ps_dw.tile([128, 128], F32, tag="dwps", name="warmps")
            nc.tensor.matmul(wps[:], id_f32[:], id_f32[:],
                             start=True, stop=True)

        def one_pass():
            # v_pack (bf16): [v(b0) 0:32 | v(b1) 32:64]
            v_pack = packp.tile([C2, HW], BF16, tag="v_pack")
            # rt (f32): h-pooled [q(b0)|q(b1) sums 0:64 | k(b0)|k(b1) max 64:128]
            rt = packp.tile([128, H * (W // 2)], F32, tag="rt")
            # vertical-pooled: qf[0:32]=b0, [32:64]=b1; kf[64:96]=b0, [96:128]=b1
            qkpool = poolt.tile([128, NP], F32, tag="qkpool")
            qf = qkpool[0:C2, :]
            kf = qkpool
            # packed transposed pools: chunk j2 cols [j2*64, j2*64+64) =
            # [T(b0) 32 | T(b1) 32], bf16 for fast qk matmuls
            qfT = attnp.tile([128, (NP // 128) * C2], F32, tag="qfT")
            kfT = attnp.tile([128, (NP // 128) * C2], F32, tag="kfT")
            # packed qk accumulator [64,64]: diag blocks = per-batch qk[d,c];
            # off-diag blocks are cross-batch garbage (ignored). One group.
            qkt = ps_sm.tile([C2, C2], F32, tag="qkt")

            xin_tiles = {}
            xb_tiles = {}

            xs_tiles = {}

            def cast_x1_strip(s):
                xb = x1p.tile([128, (R + 2) * W], BF16, tag="xb",
                              name="xb", bufs=3)
                xb_tiles[s] = xb
                xin, rs, nrows = xin_tiles.pop(s)
                if s == 0:
                    nc.gpsimd.memset(xb[:, 0:W], 0.0)
                if s == NSTRIPS - 1:
                    nc.gpsimd.memset(xb[:, (R + 1) * W:(R + 2) * W], 0.0)
                nc.vector.tensor_copy(xb[:, rs * W:(rs + nrows) * W],
                                      xin[:, rs * W:(rs + nrows) * W])
                if s in DVE_STRIPS:
                    # padded-row shifted copy: xs3[:, r, 1+w] = xb[r, w] with
                    # zero columns 0 and 129: dx=+-1 taps read
                    # xs3[:, r, 1+dx : 129+dx] at even element offsets.
                    xs = x1p.tile([128, (R + 2) * WP], BF16, tag="xs",
                                  name="xs", bufs=1)
                    xs_tiles[s] = xs
                    xs3 = xs.rearrange("p (r w) -> p r w", w=WP)
                    xbr = xb.rearrange("p (r w) -> p r w", w=W)
                    nc.scalar.copy(xs3[:, :, 1:W + 1], xbr[:])
                    nc.gpsimd.memset(xs3[:, :, 0:1], 0.0)
                    nc.gpsimd.memset(xs3[:, :, WP - 1:WP], 0.0)

            def load_x1_strip(s):
                y0 = s * R
                ys = max(y0 - 1, 0)
                ye = min(y0 + R + 1, H)
                rs = 0 if s > 0 else 1
                nrows = ye - ys

                # contiguous (unpadded) strip rows: 1 descriptor per
                # partition per image -> cheap DGE + full DMA spread
                xin = x1p.tile([128, (R + 2) * W], F32, tag="xin",
                               name="xin", bufs=2)
                xin_tiles[s] = (xin, rs, nrows)
                nc.sync.dma_start(
                    xin[0:C2, rs * W:(rs + nrows) * W],
                    x[0, 0:C2, ys:ye, :].rearrange("c h w -> c (h w)"))
                nc.sync.dma_start(
                    xin[C2:128, rs * W:(rs + nrows) * W],
                    x[1, 0:C2, ys:ye, :].rearrange("c h w -> c (h w)"))

            ox_tiles = {}

            def compute_dw_chunk(s, q):
                # one 512-col (4-row) dwconv chunk of strip s: 9 accumulating
                # matmuls + gelu evac; DMA the strip out after its 4th chunk.
                xb = xb_tiles[s]
                xb3 = xb.rearrange("p (r w) -> p r w", w=W)
                if q == 0:
                    ox_tiles[s] = x1p.tile([128, R * W], F32, tag="ox1",
                                           name="ox1", bufs=2)
                ox1 = ox_tiles[s]
                if s in DVE_STRIPS:
                    xs3 = xs_tiles[s].rearrange("p (r w) -> p r w", w=WP)
                    acc = x1p.tile([128, NCH], BF16, tag="dacc", name="dacc",
                                   bufs=2)
                    for i, (dy, dx) in enumerate(TAPS):
                        r0 = 1 + q * RPC + dy
                        if dx == 0:
                            srcv = xb[:, r0 * W:r0 * W + NCH]
                        else:
                            srcv = xs3[:, r0:r0 + RPC, 1 + dx:1 + dx + W]
                        wcol = w9[:, i:i + 1]
                        if i == 0:
                            nc.vector.tensor_scalar_mul(acc[:], srcv, wcol)
                        else:
                            tmp = x1p.tile([128, NCH], BF16, tag="dtmp",
                                           name="dtmp", bufs=1)
                            nc.vector.tensor_scalar_mul(tmp[:], srcv, wcol)
                            nc.vector.tensor_add(acc[:], acc[:], tmp[:])
                    nc.scalar.activation(ox1[:, q * NCH:(q + 1) * NCH],
                                         acc[:], ACTF.Gelu,
                                         bias=dwb_t[:, 0:1])
                else:
                    ps = ps_dw.tile([128, NCH], F32, tag="dwps", name="dwps")
                    ps3 = ps.rearrange("p (r w) -> p r w", w=W)
                    order = sorted(range(9), key=lambda t: TAPS[t] != (0, 0))
                    for i, t in enumerate(order):
                        dy, dx = TAPS[t]
                        r0 = 1 + q * RPC + dy
                        lo, hi = max(0, -dx), W + min(0, -dx)
                        nc.tensor.matmul(
                            ps3[:, :, lo:hi],
                            dw_diag[:, t * 128:(t + 1) * 128],
                            xb3[:, r0:r0 + RPC, lo + dx:hi + dx],
                            start=(i == 0), stop=(i == 8))
                    nc.scalar.activation(ox1[:, q * NCH:(q + 1) * NCH],
                                         ps[:], ACTF.Gelu, bias=dwb_t[:, 0:1])
                if q == R // RPC - 1:
                    if s + 1 in xin_tiles:
                        cast_x1_strip(s + 1)
                    del xb_tiles[s]
                    del ox_tiles[s]
                    xs_tiles.pop(s, None)
                    rng = slice(s * R * W, (s + 1) * R * W)
                    nc.scalar.dma_start(oflat[0, 0:C2, rng], ox1[0:C2, :])
                    nc.scalar.dma_start(oflat[1, 0:C2, rng], ox1[C2:128, :])

            rhs_tiles = {}

            def load_rhs_pair(pj):
                rhs = rhsp.tile([128, 2 * NCH], F32, tag="rhs",
                                name="rhs", bufs=3)
                rhs_tiles[pj] = rhs
                cols2 = slice(2 * pj * NCH, (2 * pj + 2) * NCH)
                nc.sync.dma_start(rhs[0:C2, :], xflat[0, C2:C, cols2])
                nc.sync.dma_start(rhs[C2:128, :], xflat[1, C2:C, cols2])

            def emit_attn_pool(qt):
                # vertical pool for output rows oy in [qt*16, (qt+1)*16);
                # emitted at j%8==7 so it overlaps that chunk's dw matmuls
                # and the j%8==0 transposes start dependency-free.
                oy0, oy1 = qt * (R), (qt + 1) * R
                rq = rt[0:C2, :].rearrange(
                    "p (h2 two w2) -> p h2 two w2", two=2, w2=W // 2)
                qf3 = qf.rearrange("p (h2 w2) -> p h2 w2", w2=W // 2)
                nc.gpsimd.tensor_add(qf3[:, oy0:oy1, :],
                                     rq[:, oy0:oy1, 0, :],
                                     rq[:, oy0:oy1, 1, :])
                lo = max(oy0, 1)
                nc.gpsimd.tensor_add(qf3[:, lo:oy1, :], qf3[:, lo:oy1, :],
                                     rq[:, lo - 1:oy1 - 1, 1, :])
                rk = rt[C2:128, :].rearrange(
                    "p (h2 two w2) -> p h2 two w2", two=2, w2=W // 2)
                kf3 = kf[C2:128, :].rearrange("p (h2 w2) -> p h2 w2",
                                              w2=W // 2)
                nc.vector.tensor_max(kf3[:, oy0:oy1, :],
                                     rk[:, oy0:oy1, 0, :],
                                     rk[:, oy0:oy1, 1, :])

            def emit_attn_mm(qt):
                # packed transposes ([64,128] -> [128,64]) + qk matmuls
                # for this quarter's 8 column chunks of 128
                for (src, base, dstT) in ((qf, 0, qfT), (kf, C2, kfT)):
                    ps = ps_tr.tile([128, 512], F32, tag="trps", name="trps")
                    for jj in range(8):
                        j2 = qt * 8 + jj
                        nc.tensor.transpose(
                            ps[:, jj * C2:(jj + 1) * C2],
                            src[base:base + C2, j2 * 128:(j2 + 1) * 128],
                            id_f32[base:base + C2, base:base + C2])
                    nc.vector.tensor_copy(
                        dstT[:, qt * 512:(qt + 1) * 512], ps[:])
                for jj in range(8):
                    j2 = qt * 8 + jj
                    sl = slice(j2 * C2, (j2 + 1) * C2)
                    nc.tensor.matmul(qkt[:], kfT[:, sl], qfT[:, sl],
                                     start=(qt == 0 and jj == 0),
                                     stop=(qt == 3 and jj == 7))

            # ---------- conv1x1 + dwconv chunks + attn, interleaved ----------
            load_x1_strip(0)
            load_rhs_pair(0)
            load_rhs_pair(1)
            cast_x1_strip(0)
            vl = None
            for j in range(NCHUNKS):
                cols = bass.ts(j, NCH)
                if j % 2 == 0:
                    if j // 2 + 2 < NCHUNKS // 2:
                        load_rhs_pair(j // 2 + 2)
                    rhs = rhs_tiles.pop(j // 2)
                    rhsb = rhsp.tile([128, 2 * NCH], BF16, tag="rhsb",
                                     name="rhsb", bufs=2)
                    nc.vector.tensor_copy(rhsb[:], rhs[:])
                rhsv = rhs[:, (j % 2) * NCH:(j % 2 + 1) * NCH]
                rhsvb = rhsb[:, (j % 2) * NCH:(j % 2 + 1) * NCH]

                Ap = ps_convA.tile([128, NCH], F32, tag="convA", name="convA")
                nc.tensor.matmul(Ap[:], lhsA, rhsv, start=True, stop=True)
                qg = rhsp.tile([128, NCH], F32, tag="qg")
                nc.scalar.activation(qg[:], Ap[:], ACTF.Gelu,
                                     bias=biasA[:, 0:1])

                # fused horizontal pooling into rt (both batches per op)
                rrows = rt[:, j * RPC * (W // 2):(j + 1) * RPC * (W // 2)]
                Xq = qg[0:C2, :].rearrange(
                    "p (h w2 two) -> p h w2 two", h=RPC, two=2)
                r3 = rrows[0:C2, :].rearrange("p (h w2) -> p h w2", h=RPC)
                nc.gpsimd.tensor_add(r3[:], Xq[:, :, :, 0], Xq[:, :, :, 1])
                nc.gpsimd.tensor_add(r3[:, :, 1:W // 2], r3[:, :, 1:W // 2],
                                     Xq[:, :, 0:W // 2 - 1, 1])
                Xk = qg[C2:128, :].rearrange(
                    "p (h w2 two) -> p h w2 two", h=RPC, two=2)
                m3 = rrows[C2:128, :].rearrange("p (h w2) -> p h w2", h=RPC)
                nc.vector.tensor_max(m3[:], Xk[:, :, :, 0], Xk[:, :, :, 1])

                Bp = ps_convB.tile([128, NCH], F32, tag="convB", name="convB")
                nc.tensor.matmul(Bp[:], lhsB, rhsvb, start=True, stop=True)
                # single gelu for v (rows 0:64) + l (rows 64:128); DVE casts
                # the v half to bf16, the l half is DMAd out per chunk-pair
                if j % 2 == 0:
                    vl = rhsp.tile([128, 2 * NCH], F32, tag="vl", bufs=2)
                vls = vl[:, (j % 2) * NCH:(j % 2 + 1) * NCH]
                nc.scalar.activation(vls, Bp[:], ACTF.Gelu,
                                     bias=biasB[:, 0:1])
                nc.vector.tensor_copy(v_pack[:, cols], vls[0:C2, :])
                if j % 2 == 1:
                    cols2 = slice((j - 1) * NCH, (j + 1) * NCH)
                    nc.scalar.dma_start(oflat[0, C2:96, cols2],
                                        vl[C2:96, :])
                    nc.scalar.dma_start(oflat[1, C2:96, cols2],
                                        vl[96:128, :])

                s, q = j // 4, j % 4
                if q == 0 and s + 1 < NSTRIPS:
                    load_x1_strip(s + 1)
                compute_dw_chunk(s, q)
                if j % 8 == 7:
                    emit_attn_pool(j // 8)
                if j in (8, 16, 24):
                    emit_attn_mm(j // 8 - 1)

            # ---------- attention stats + packed out2 ----------
            emit_attn_mm(3)
            qkts = attnp.tile([C2, C2], F32, tag="qkts")
            nc.vector.tensor_scalar_mul(qkts[:], qkt[:], 1.0 / 9.0)
            nmax = attnp.tile([C2, 1], F32, tag="nmax")
            ET = attnp.tile([C2, C2], F32, tag="ET")
            ssum = attnp.tile([C2, 1], F32, tag="ssum")
            rec = attnp.tile([C2, 1], F32, tag="rec")
            ETn = attnp.tile([C2, C2], F32, tag="ETn")
            nc.vector.memset(ETn[:], 0.0)
            for bi in range(BPC):
                rs = slice(C4 * bi, C4 * bi + C4)
                blk = (rs, slice(C4 * bi, C4 * bi + C4))
                nc.vector.tensor_reduce(nmax[rs, :], qkts[blk], axis=AX.X,
                                        op=ALU.max, negate=True)
                nc.scalar.activation(ET[blk], qkts[blk], ACTF.Exp,
                                     bias=nmax[rs, 0:1])
                nc.vector.reduce_sum(ssum[rs, :], ET[blk], axis=AX.X)
                nc.vector.reciprocal(rec[rs, :], ssum[rs, :])
                nc.vector.tensor_scalar_mul(ETn[blk], ET[blk], rec[rs, 0:1])
            etp = ps_sm.tile([C2, C2], F32, tag="qkt", name="etp")
            nc.tensor.transpose(etp[:], ETn[:], id_f32[0:C2, 0:C2])
            E_pack = attnp.tile([C2, C2], BF16, tag="E_pack")
            nc.vector.tensor_copy(E_pack[:], etp[:])

            # out2: one packed matmul per chunk -> [64,512] psum
            # (partitions p=(b*32+d)); DVE evac; one DMA per 2 chunks.
            st = None
            for j in range(NCHUNKS):
                cols = bass.ts(j, NCH)
                pool = ps_convA if j % 2 == 0 else ps_convB
                tag = "convA" if j % 2 == 0 else "convB"
                o2 = pool.tile([C2, NCH], F32, tag=tag, name="o2ps")
                nc.tensor.matmul(o2[:], E_pack[:], v_pack[:, cols],
                                 start=True, stop=True)
                if j % 2 == 0:
                    st = stgp.tile([C2, 2 * NCH], F32, tag="st", bufs=3)
                dst = st[:, (j % 2) * NCH:(j % 2 + 1) * NCH]
                if j % 2 == 0:
                    nc.vector.tensor_copy(dst, o2[:])
                else:
                    nc.scalar.copy(dst, o2[:])
                if j % 2 == 1:
                    cols2 = slice((j - 1) * NCH, (j + 1) * NCH)
                    nc.gpsimd.dma_start(oflat[0, 96:128, cols2],
                                        st[0:C4, :])
                    nc.gpsimd.dma_start(oflat[1, 96:128, cols2],
                                        st[C4:C2, :])

        for _ in range(loops):
            one_pass()

    nc.compile()
    return nc


def prep_params(dw_w, dw_b, qkvl_w, qkvl_b):
    """Host-side packing of the small parameter tensors into the layouts the
    kernel consumes (block-diagonal conv weights, batch-replicated biases,
    9 diagonal dwconv blocks)."""
    f = np.float32
    qwT = np.ascontiguousarray(
        np.asarray(qkvl_w, f).reshape(C, C2).T)          # [64ic, 128oc]
    lhsA = np.zeros((128, 128), f)
    lhsB = np.zeros((128, 128), f)
    lhsA[0:C2, 0:C4] = qwT[:, 0:C4]
    lhsA[C2:128, C4:C2] = qwT[:, 0:C4]
    lhsA[0:C2, C2:96] = qwT[:, C4:C2]
    lhsA[C2:128, 96:128] = qwT[:, C4:C2]
    lhsB[0:C2, 0:C4] = qwT[:, C2:96]
    lhsB[C2:128, C4:C2] = qwT[:, C2:96]
    lhsB[0:C2, C2:96] = qwT[:, 96:128]
    lhsB[C2:128, 96:128] = qwT[:, 96:128]
    qb = np.asarray(qkvl_b, f)
    biasA = np.concatenate([qb[0:C4], qb[0:C4], qb[C4:C2], qb[C4:C2]])
    biasB = np.concatenate([qb[C2:96], qb[C2:96], qb[96:128], qb[96:128]])
    dwb = np.tile(np.asarray(dw_b, f), 2)
    w9 = np.asarray(dw_w, f).reshape(C2, 9)
    wfull = np.vstack([w9, w9])                          # [128, 9]
    dwdiag = np.zeros((128, 9, 128), f)
    for t in range(9):
        dwdiag[np.arange(128), t, np.arange(128)] = wfull[:, t]
    bf = ml_dtypes.bfloat16
    return {
        "lhsA": np.ascontiguousarray(lhsA),
        "lhsB": np.ascontiguousarray(lhsB.astype(bf)),
        "biases": np.ascontiguousarray(
            np.stack([biasA, biasB, dwb], axis=1)),
        "dwdiag": np.ascontiguousarray(
            dwdiag.reshape(128, 9 * 128).astype(bf)),
        "dww9": np.ascontiguousarray(wfull),
    }


_NC_CACHE = None


def _get_nc():
    global _NC_CACHE
    if _NC_CACHE is None:
        _NC_CACHE = build_nc()
    return _NC_CACHE


def kernel(x, dw_w, dw_b, qkvl_w, qkvl_b):
    x = np.ascontiguousarray(np.asarray(x, dtype=np.float32))
    shared = prep_params(dw_w, dw_b, qkvl_w, qkvl_b)
    nc = _get_nc()
    in_maps = [
        {"x": x[c * BPC:(c + 1) * BPC], **shared} for c in range(N_CORES)
    ]
    res = bass_utils.run_bass_kernel_spmd(nc, in_maps,
                                          core_ids=list(range(N_CORES)))
    return np.concatenate([res.results[c]["out"] for c in range(N_CORES)],
                          axis=0)
